# revision 10
# baseline (speedup 1.0000x reference)
"""Trainium2 Bass kernel for batched windowed DFT, v5: two-stage hop-block DFT.

Per core (one batch row): frames share their hop-size-512 blocks, so the DFT
is computed per BLOCK and frames are assembled from block spectra:
  X_t[k] = sum_{c=0..3} (-i)^{kc} G_{t+c}[k],  G_j = DFT_2048 of block j.
The twiddles are exactly {1,-i,-1,i}; grouping bins by k mod 4 (one class per
128-row tile) and using DFT linearity, the pairwise partials are GEMMs of
signal combinations computed on the DVE:
  class 0: S = DFT(U), U_j = b_j + b_{j+1};  X[t] = S[t] + S[t+2]
  class 2: S = DFT(V), V_j = b_j - b_{j+1};  X[t] = S[t] + S[t+2]
  class 1/3: E+iF = DFT(D), D_j = b_j - b_{j+2};
     cls1: X_re = E[t] + F[t+1], X_im = F[t] - E[t+1]
     cls3: X_re = E[t] - F[t+1], X_im = F[t] + E[t+1]
Stage 1 (PE): 64 matmuls, contraction 512, free dim 510/511 — one PSUM bank
per group, no ragged tails. Stage 2: 16 single-PSUM-operand evacuations
(DVE/Act) + 16 plain SBUF adds (Pool).
Host: Hann window as the 3-tap frequency stencil 0.5X[k]-0.25(X[k-1]+X[k+1]),
frames 509..512, bin 512, and the Hermitian half — O(output) marshalling.
Device bins: 8 tiles of 128 = {0..1024} minus 512; frames 0..508.
"""
import numpy as np

import concourse.bacc as bacc
import concourse.mybir as mybir
import concourse.tile as tile

F32 = mybir.dt.float32
F16 = mybir.dt.float16
N_CORES = 8
TD = 509          # frames on device (509..512 are host epsilon-columns)
ST = 512          # stage/out column stride per tile
NU = 511          # U/V variant columns (blocks j and j+1)
ND = 510          # D variant columns (blocks j and j+2)
# tile -> first bin (step 4); classes 1/3 first so the tail is one short chain
TILE_B0 = [1, 3, 513, 515, 0, 2, 516, 514]
TILE_CLASS = [b % 4 for b in TILE_B0]        # [1, 3, 1, 3, 0, 2, 0, 2]


def build_nc(reps=1):
    nc = bacc.Bacc("TRN2", target_bir_lowering=False, debug=False,
                   num_devices=N_CORES)
    sig_d = nc.dram_tensor("sig", [128, 4 * 512], F16, kind="ExternalInput")
    wt_d = nc.dram_tensor("wt", [128, 8192], F16, kind="ExternalInput")
    outR_d = nc.dram_tensor("outR", [128, 8 * ST], F16, kind="ExternalOutput")
    outI_d = nc.dram_tensor("outI", [128, 8 * ST], F16, kind="ExternalOutput")

    with tile.TileContext(nc) as tc:
        with (
            tc.tile_pool(name="sigp", bufs=1) as sigp,
            tc.tile_pool(name="wts", bufs=1) as wts,
            tc.tile_pool(name="vart", bufs=1) as vart,
            tc.tile_pool(name="evp", bufs=4) as evp,
            tc.tile_pool(name="stg", bufs=1) as stg,
            tc.tile_pool(name="ps", bufs=4, space="PSUM") as ps,
        ):
            XB = sigp.tile([128, 4 * 512], F16, tag="XB")
            W = wts.tile([128, 8192], F16, tag="W")
            VU = vart.tile([128, 4 * NU], F16, tag="VU")
            VV = vart.tile([128, 4 * NU], F16, tag="VV")
            VD = vart.tile([128, 4 * ND], F16, tag="VD")
            stR = stg.tile([128, 8 * ST], F16, tag="stR")
            stI = stg.tile([128, 8 * ST], F16, tag="stI")
            nc.gpsimd.memset(stR[:], 0.0)
            nc.gpsimd.memset(stI[:], 0.0)

            import contextlib
            rep_cm = tc.For_i(0, reps) if reps > 1 else contextlib.nullcontext()
            with rep_cm:
                nc.sync.dma_start(W[:, 0:1024], wt_d.ap()[:, 0:1024])
                for a in range(4):
                    asl = slice(a * 512, (a + 1) * 512)
                    eng = nc.scalar if a % 2 == 0 else nc.gpsimd
                    eng.dma_start(XB[:, asl], sig_d.ap()[:, asl])
                for c in range(1, 8):
                    nc.sync.dma_start(W[:, c * 1024:(c + 1) * 1024],
                                      wt_d.ap()[:, c * 1024:(c + 1) * 1024])
                # signal variants per chunk (DVE); D first: tiles 0-3 use it
                for a in range(4):
                    b0 = a * 512
                    nc.vector.tensor_sub(VD[:, a * ND:(a + 1) * ND],
                                         XB[:, b0:b0 + ND], XB[:, b0 + 2:b0 + 2 + ND])
                for a in range(4):
                    b0 = a * 512
                    nc.vector.tensor_add(VU[:, a * NU:(a + 1) * NU],
                                         XB[:, b0:b0 + NU], XB[:, b0 + 1:b0 + 1 + NU])
                    nc.vector.tensor_sub(VV[:, a * NU:(a + 1) * NU],
                                         XB[:, b0:b0 + NU], XB[:, b0 + 1:b0 + 1 + NU])

                for T in range(8):
                    cls = TILE_CLASS[T]
                    if cls in (1, 3):
                        var, NC = VD, ND
                    elif cls == 0:
                        var, NC = VU, NU
                    else:
                        var, NC = VV, NU
                    accR = ps.tile([128, NU], F32, tag="accR")
                    accI = ps.tile([128, NU], F32, tag="accI")
                    for a in range(4):
                        for comp in (1, 0):   # im first: its partial drains early
                            acc = accI if comp else accR
                            wc0 = ((T * 2 + comp) * 4 + a) * 128
                            nc.tensor.matmul(acc[:, 0:NC], W[:, wc0:wc0 + 128],
                                             var[:, a * NC:(a + 1) * NC],
                                             start=(a == 0), stop=(a == 3))
                    # evacuate partials (single PSUM operand each)
                    sE = evp.tile([128, NU], F16, tag="sE")
                    sF = evp.tile([128, NU], F16, tag="sF")
                    nc.scalar.copy(sF[:, 0:NC], accI[:, 0:NC])
                    nc.vector.tensor_copy(sE[:, 0:NC], accR[:, 0:NC])
                    col = slice(T * ST, T * ST + TD)
                    if cls in (0, 2):
                        nc.gpsimd.tensor_add(stI[:, col], sF[:, 0:TD], sF[:, 2:TD + 2])
                        nc.gpsimd.tensor_add(stR[:, col], sE[:, 0:TD], sE[:, 2:TD + 2])
                    elif cls == 1:
                        # X_re = E[t] + F[t+1]; X_im = F[t] - E[t+1]
                        nc.gpsimd.tensor_add(stR[:, col], sE[:, 0:TD], sF[:, 1:TD + 1])
                        nc.gpsimd.tensor_sub(stI[:, col], sF[:, 0:TD], sE[:, 1:TD + 1])
                    else:
                        # X_re = E[t] - F[t+1]; X_im = F[t] + E[t+1]
                        nc.gpsimd.tensor_sub(stR[:, col], sE[:, 0:TD], sF[:, 1:TD + 1])
                        nc.gpsimd.tensor_add(stI[:, col], sF[:, 0:TD], sE[:, 1:TD + 1])
                    if T in (3, 5, 6, 7):
                        q0 = {3: 0, 5: 4, 6: 6, 7: 7}[T] * ST
                        qcol = slice(q0, (T + 1) * ST)
                        nc.scalar.dma_start(outR_d.ap()[:, qcol], stR[:, qcol])
                        nc.sync.dma_start(outI_d.ap()[:, qcol], stI[:, qcol])
    nc.compile()
    return nc


def host_prep(x, wsin, wcos):
    """Marshal full inputs into per-core input maps."""
    x = np.asarray(x, dtype=np.float32)
    B = x.shape[0]
    xp = np.pad(x, ((0, 0), (1024, 1024)), mode="reflect")
    st = xp.strides
    xb = np.lib.stride_tricks.as_strided(
        xp, (B, 512, 512), (st[0], 512 * st[1], st[1]))   # [b, block j, r]
    sig = np.ascontiguousarray(
        xb.reshape(B, 512, 4, 128).transpose(0, 3, 2, 1).reshape(B, 128, 4 * 512)
    ).astype(np.float16)

    r_ = np.arange(512, dtype=np.float64)
    wt = np.zeros((128, 8192), np.float16)
    for T in range(8):
        kq = (TILE_B0[T] + 4 * np.arange(128)).astype(np.float64)
        ang = 2.0 * np.pi * np.outer(r_, kq) / 2048.0          # [r, q]
        for comp in range(2):
            vals = np.cos(ang) if comp == 0 else -np.sin(ang)
            for a in range(4):
                c0 = ((T * 2 + comp) * 4 + a) * 128
                wt[:, c0:c0 + 128] = vals[a * 128:(a + 1) * 128, :]
    return [{"sig": sig[b], "wt": wt} for b in range(B)]


def assemble(results, x, wsin, wcos):
    """Host: frames 509..512, bin 512, Hann stencil, Hermitian half."""
    x = np.asarray(x, dtype=np.float32)
    B = len(results)
    xp = np.pad(x, ((0, 0), (1024, 1024)), mode="reflect")
    st = xp.strides
    XR = np.zeros((B, 1025, 513), np.float32)
    XI = np.zeros((B, 1025, 513), np.float32)
    for b in range(B):
        oR = results[b]["outR"].astype(np.float32).reshape(128, 8, ST)
        oI = results[b]["outI"].astype(np.float32).reshape(128, 8, ST)
        for T in range(8):
            bins = TILE_B0[T] + 4 * np.arange(128)
            XR[b, bins, :TD] = oR[:, T, :TD]
            XI[b, bins, :TD] = oI[:, T, :TD]
    # bin 512, t < TD: X[512,t] = sum_n f e^{-i pi n/2} (pattern-strided sums)
    fr = np.lib.stride_tricks.as_strided(
        xp, (B, 513, 2048), (st[0], 512 * st[1], st[1]))
    frd = fr[:, :TD]
    XR[:, 512, :TD] = frd[:, :, 0::4].sum(2) - frd[:, :, 2::4].sum(2)
    XI[:, 512, :TD] = -(frd[:, :, 1::4].sum(2) - frd[:, :, 3::4].sum(2))
    # frames 509..512: unwindowed X for all bins 0..1024 via host GEMM
    kk = np.arange(1025, dtype=np.float64)
    ang = 2.0 * np.pi * np.outer(kk, np.arange(2048, dtype=np.float64)) / 2048.0
    ftail = fr[:, TD:513].astype(np.float64)                 # [B, 4, 2048]
    XR[:, :, TD:513] = np.einsum("btn,kn->bkt", ftail, np.cos(ang)).astype(np.float32)
    XI[:, :, TD:513] = np.einsum("btn,kn->bkt", ftail, -np.sin(ang)).astype(np.float32)
    # window stencil: Xwin[k] = 0.5X[k] - 0.25(X[k-1]+X[k+1]); X[-1]=conj X[1],
    # X[1025]=conj X[1023]
    XRm1 = np.concatenate([XR[:, 1:2], XR[:, :-1]], axis=1)
    XIm1 = np.concatenate([-XI[:, 1:2], XI[:, :-1]], axis=1)
    XRp1 = np.concatenate([XR[:, 1:], XR[:, 1023:1024]], axis=1)
    XIp1 = np.concatenate([XI[:, 1:], -XI[:, 1023:1024]], axis=1)
    WR = 0.5 * XR - 0.25 * (XRm1 + XRp1)
    WI = 0.5 * XI - 0.25 * (XIm1 + XIp1)
    R = np.empty((B, 2048, 513), np.float32)
    I = np.empty((B, 2048, 513), np.float32)
    R[:, :1025] = WR
    I[:, :1025] = WI
    R[:, 1025:] = R[:, 1023:0:-1]
    I[:, 1025:] = -I[:, 1023:0:-1]
    return R, I


class _Runner:
    """Build once, jit once, run many (shard_map over the 8 cores)."""

    def __init__(self, reps=1):
        import jax
        from jax.sharding import Mesh, PartitionSpec
        from jax.experimental.shard_map import shard_map
        from concourse.bass2jax import _bass_exec_p, install_neuronx_cc_hook

        install_neuronx_cc_hook()
        self.jax = jax
        nc = build_nc(reps=reps)
        self.nc = nc
        in_names, out_names, out_avals = [], [], []
        for alloc in nc.m.functions[0].allocations:
            if not isinstance(alloc, mybir.MemoryLocationSet):
                continue
            name = alloc.memorylocations[0].name
            if alloc.kind == "ExternalInput":
                in_names.append(name)
            elif alloc.kind == "ExternalOutput":
                out_names.append(name)
                out_avals.append(jax.core.ShapedArray(
                    tuple(alloc.tensor_shape), mybir.dt.np(alloc.dtype)))
        self.in_names, self.out_names, self.out_avals = in_names, out_names, out_avals
        n_params = len(in_names)
        all_names = in_names + out_names

        def _body(*args):
            outs = _bass_exec_p.bind(
                *args,
                out_avals=tuple(out_avals),
                in_names=tuple(all_names),
                out_names=tuple(out_names),
                lowering_input_output_aliases=(),
                sim_require_finite=True,
                sim_require_nnan=True,
                nc=nc,
            )
            return tuple(outs)

        devices = jax.devices()[:N_CORES]
        mesh = Mesh(np.asarray(devices), ("core",))
        n_outs = len(out_names)
        self._fn = jax.jit(
            shard_map(_body, mesh=mesh,
                      in_specs=(PartitionSpec("core"),) * (n_params + n_outs),
                      out_specs=(PartitionSpec("core"),) * n_outs,
                      check_rep=False),
            keep_unused=True,
        )
        self._zeros = [np.zeros((N_CORES * a.shape[0], *a.shape[1:]), a.dtype)
                       for a in out_avals]

    def prepare(self, in_maps):
        pid = self.nc.partition_id_tensor.name if self.nc.partition_id_tensor else None
        in_maps = [
            dict(m, **({pid: np.array([[c]], dtype=np.uint32)} if pid else {}))
            for c, m in enumerate(in_maps)
        ]
        concat = [np.concatenate([np.asarray(m[name]) for m in in_maps], axis=0)
                  for name in self.in_names]
        self._args = [self.jax.device_put(a) for a in concat + self._zeros]
        self.jax.block_until_ready(self._args)

    def run(self):
        out = self._fn(*self._args)
        self.jax.block_until_ready(out)
        return out

    def results(self, out):
        res = []
        for c in range(N_CORES):
            d = {}
            for i, name in enumerate(self.out_names):
                a = np.asarray(out[i])
                d[name] = a.reshape(N_CORES, *self.out_avals[i].shape)[c]
            res.append(d)
        return res


_RUNNER = None


def kernel(x, wsin, wcos):
    """Full inputs in, full output out: returns (real, -imag) as in reference."""
    global _RUNNER
    if _RUNNER is None:
        _RUNNER = _Runner(reps=1)
    ins = host_prep(x, wsin, wcos)
    _RUNNER.prepare(ins)
    out = _RUNNER.run()
    R, I = assemble(_RUNNER.results(out), x, wsin, wcos)
    return R, I
